# revision 21
# baseline (speedup 1.0000x reference)
"""Trainium2 Bass kernel for DepthFFN (histogram_binning).

Computes, for inputs
  image_features  (2, 32, 47, 156)  f32
  depth_logits    (2, 121, 47, 156) f32
  depth_maps      (2, 376, 1248)    f32
  depth_target_bin(2, 47, 156)      i32
the reference outputs
  frustum_features        (2, 32, 120, 47, 156) = softmax(logits)[:, :120] x image_features
  frustum_features_target (2, 32, 120, 47, 156) = onehot(bin)[:, :120]    x image_features
  pooled_depth            (2, 47, 156)          = sparse 8x8 avg pool of depth_maps

Sharding: 8 cores = (batch b in {0,1}) x (depth chunk dc in {0..3}, 30 bins
each). Each core writes its [30, 32, 7332] d-major slice of both frustum
tensors, and redundantly computes the (tiny) pooled output; the host takes
core 0's copy.

Structure: the hw = 7332 free axis is cut into 4 pipeline sections
(3 x 2048 + 1188); every step is column-local so sections overlap through
the Tile scheduler. All matmuls are bf16 (full PE rate):
  - exp = Exp(logits) is split into bf16 hi + lo parts (hi+lo carries
    ~17 mantissa bits; everything downstream accumulates in fp32 PSUM);
  - softmax sum = ones^T (exp_hi + exp_lo) (PE); exact reciprocal via a
    DRAM-bounce reshape onto ~64 partitions (DVE); the 1/sum row is folded
    into a scaled image (img * 1/sum broadcast over 32 partitions by a
    replicating DMA), never into the probs;
  - one-hot(bin) [121, w] built once per section by two ACT passes
    (Relu(1 - Abs(bin_row - k_partition))), exact 0/1 in bf16;
  - per d-tile (4 depth bins x 32 channels on 128 partitions): one bf16
    selection matmul replicates exp hi+lo rows (features) and one-hot rows
    (target) into PSUM; frustum tile = psum * img_scaled_rep (DVE);
    target tile = psum * img_rep, split between DVE (PSUM direct) and
    ACT-copy + GpSimd (SBUF) to balance engine load;
  - output stores alternate between the two HWDGE queues (Sync and Scalar
    sequencers) to halve per-queue DMA dispatch occupancy.
"""

import numpy as np
import ml_dtypes

import concourse.bacc as bacc
import concourse.bass as bass
import concourse.mybir as mybir
import concourse.tile as tile
from concourse.bass_utils import run_bass_kernel_spmd

F32 = mybir.dt.float32
BF16 = mybir.dt.bfloat16
AF = mybir.ActivationFunctionType
OP = mybir.AluOpType
AX = mybir.AxisListType

B, C, D, DP1 = 2, 32, 120, 121
H, W = 47, 156
HW = H * W  # 7332
ND = 30  # depth bins per core
NCORES = 8
CW = 512  # matmul moving-operand chunk (one fp32 PSUM bank)
PSW = 512  # PSUM tile width (1 bank -> 8 pipeline slots)
SECW = 2048  # pipeline section width
SECS = [(s, min(SECW, HW - s)) for s in range(0, HW, SECW)]
# per-section reshape factors for the reciprocal bounce (rows x cols = w)
RECIP_SHAPE = {2048: (64, 32), 1188: (44, 27)}
NT = 8  # d-tiles per core: 7 x (4 bins x 32 ch) + 1 x (2 bins x 32 ch)
PQ = 312  # pooling quarter width (in depth_maps columns)
# target-tile route: per (t, section), True -> GpSimd path, False -> DVE path.
# ~3/4 on GpSimd frees the DVE for the frustum stream.
GP_ROUTE = [t % 2 == 0 for t in range(NT)]


def build_program():
    nc = bacc.Bacc(
        "TRN2",
        target_bir_lowering=False,
        debug=False,
        num_devices=NCORES,
    )

    img_d = nc.dram_tensor("img", [C, HW], F32, kind="ExternalInput").ap()
    logits_d = nc.dram_tensor("logits", [DP1, HW], F32, kind="ExternalInput").ap()
    binf16_d = nc.dram_tensor("binf16", [1, HW], BF16, kind="ExternalInput").ap()
    negk_d = nc.dram_tensor("negk", [DP1, 1], F32, kind="ExternalInput").ap()
    sel_d = nc.dram_tensor("sel", [DP1, ND * C], BF16, kind="ExternalInput").ap()
    onescolf_d = nc.dram_tensor("onescolf", [DP1, 1], F32, kind="ExternalInput").ap()
    dmaps_d = nc.dram_tensor("dmaps", [94, 8, 1248], F32, kind="ExternalInput").ap()

    out_f_d = nc.dram_tensor("out_f", [ND * C, HW], F32, kind="ExternalOutput").ap()
    out_t_d = nc.dram_tensor("out_t", [ND * C, HW], F32, kind="ExternalOutput").ap()
    pooled_d = nc.dram_tensor("pooled", [94, 156], F32, kind="ExternalOutput").ap()

    with tile.TileContext(nc) as tc:
        with (
            tc.tile_pool(name="const", bufs=1) as constp,
            tc.tile_pool(name="sec", bufs=2) as secp,
            tc.tile_pool(name="outf", bufs=4) as outfp,
            tc.tile_pool(name="outt", bufs=4) as outtp,
            tc.tile_pool(name="psum", bufs=8, space="PSUM") as psp,
            tc.tile_pool(name="dram", bufs=1, space="DRAM") as dramp,
            tc.tile_pool(name="poolx", bufs=1) as poolxp,
        ):
            # ---- constants ----
            sel_s = constp.tile([DP1, ND * C], BF16, tag="sel")
            nc.sync.dma_start(sel_s, sel_d)
            onescolf_s = constp.tile([DP1, 1], F32, tag="onescolf")
            nc.sync.dma_start(onescolf_s, onescolf_d)
            negk_s = constp.tile([DP1, 1], F32, tag="negk")
            nc.sync.dma_start(negk_s, negk_d)

            sums_b = dramp.tile([1, HW], F32, tag="sums")
            inv_b = dramp.tile([1, HW], F32, tag="inv")

            pooled_s = poolxp.tile([94, 156], F32, tag="pooled")
            svacc = poolxp.tile([94, 156], F32, tag="svacc")
            smacc = poolxp.tile([94, 156], F32, tag="smacc")

            def prologue(si):
                c0, w = SECS[si]
                sec = slice(c0, c0 + w)
                pq = [(q, min(PSW, w - q)) for q in range(0, w, PSW)]

                # ---- prologue: exp, hi/lo split, softmax sum, reciprocal ----
                exp_s = secp.tile([DP1, SECW], F32, tag="exp")
                nc.sync.dma_start(exp_s[:, :w], logits_d[:, sec])
                nc.scalar.activation(exp_s[:, :w], exp_s[:, :w], AF.Exp)
                phi_s = secp.tile([DP1, SECW], BF16, tag="phi")
                # f32 -> bf16 rounding done by the DMA cast path (SWDGE)
                nc.gpsimd.dma_start(phi_s[:, :w], exp_s[:, :w])
                plo_s = secp.tile([DP1, SECW], BF16, tag="plo")
                nc.gpsimd.tensor_tensor(
                    plo_s[:, :w], exp_s[:, :w], phi_s[:, :w], OP.subtract
                )

                sum_s = secp.tile([1, SECW], F32, tag="sumsec", bufs=1)
                for q, qw in pq:
                    ps = psp.tile([128, PSW], F32, tag="ps")
                    for k in range(0, qw, CW):
                        kw = min(CW, qw - k)
                        nc.tensor.matmul(
                            ps[:1, k : k + kw],
                            onescolf_s,
                            exp_s[:, q + k : q + k + kw],
                            start=True,
                            stop=True,
                        )
                    nc.scalar.copy(sum_s[:, q : q + qw], ps[:1, :qw])
                nc.sync.dma_start(sums_b[:, sec], sum_s[:, :w])

                rr, rc = RECIP_SHAPE[w]
                r_s = secp.tile([rr, rc], F32, tag="rsec")
                nc.sync.dma_start(
                    r_s, sums_b[:, sec].rearrange("o (p q) -> (o p) q", q=rc)
                )
                nc.vector.reciprocal(r_s, r_s)
                nc.sync.dma_start(
                    inv_b[:, sec].rearrange("o (p q) -> (o p) q", q=rc), r_s
                )

                # ---- image: raw + softmax-scaled, each replicated 4x,
                # built directly inside the replicated tiles ----
                irep_s = secp.tile([128, SECW], F32, tag="irep")
                isrep_s = secp.tile([128, SECW], F32, tag="isrep")
                nc.sync.dma_start(irep_s[:C, :w], img_d[:, sec])
                nc.sync.dma_start(
                    isrep_s[:C, :w],
                    bass.AP(inv_b.tensor, inv_b.offset + c0, [[0, C], [1, w]]),
                )
                nc.gpsimd.tensor_tensor(
                    isrep_s[:C, :w], isrep_s[:C, :w], irep_s[:C, :w], OP.mult
                )
                for k in range(1, 4):
                    nc.sync.dma_start(
                        irep_s[32 * k : 32 * (k + 1), :w], irep_s[:C, :w]
                    )
                    nc.sync.dma_start(
                        isrep_s[32 * k : 32 * (k + 1), :w], isrep_s[:C, :w]
                    )

                # ---- one-hot(bin) [121, w], exact 0/1 in bf16 ----
                brep_s = secp.tile([DP1, SECW], BF16, tag="brep", bufs=1)
                nc.scalar.dma_start(
                    brep_s[:, :w],
                    bass.AP(
                        binf16_d.tensor, binf16_d.offset + c0, [[0, DP1], [1, w]]
                    ),
                )
                oh_s = secp.tile([DP1, SECW], BF16, tag="oh")
                # |bin - k|, then relu(1 - |bin - k|): 1 iff bin == k
                nc.scalar.activation(
                    oh_s[:, :w], brep_s[:, :w], AF.Abs, bias=negk_s, scale=1.0
                )
                nc.scalar.activation(
                    oh_s[:, :w], oh_s[:, :w], AF.Relu, bias=1.0, scale=-1.0
                )
                # ---- pooling quarter: rows [2*si, 2*si+2) of each 8x8
                # block (contiguous 2x1248 floats per partition), accumulated
                # across sections ----
                xp = poolxp.tile([94, 2, 1248], F32, tag="xp", bufs=1)
                nc.sync.dma_start(xp, dmaps_d[:, 2 * si : 2 * si + 2, :])
                mkp = poolxp.tile([94, 2, 1248], F32, tag="mkp", bufs=1)
                # depth values are >= 0, so Sign(x) == (x != 0)
                nc.scalar.activation(mkp, xp, AF.Sign)
                svq = svacc if si == 0 else poolxp.tile(
                    [94, 156], F32, tag="svq", bufs=2
                )
                smq = smacc if si == 0 else poolxp.tile(
                    [94, 156], F32, tag="smq", bufs=2
                )
                nc.vector.tensor_reduce(
                    svq,
                    xp.rearrange("p r (j q) -> p j r q", q=8),
                    axis=AX.XY,
                    op=OP.add,
                )
                nc.vector.tensor_reduce(
                    smq,
                    mkp.rearrange("p r (j q) -> p j r q", q=8),
                    axis=AX.XY,
                    op=OP.add,
                )
                if si > 0:
                    nc.vector.tensor_add(svacc, svacc, svq)
                    nc.vector.tensor_add(smacc, smacc, smq)

                return dict(
                    c0=c0, w=w, sec=sec, pq=pq, phi=phi_s, plo=plo_s,
                    irep=irep_s, isrep=isrep_s, oh=oh_s
                )

            def main(st):
                sec, pq, w = st["sec"], st["pq"], st["w"]
                phi_s, plo_s = st["phi"], st["plo"]
                irep_s, isrep_s, oh_s = st["irep"], st["isrep"], st["oh"]
                for t in range(NT):
                    pt = 128 if t < NT - 1 else 64
                    m0 = 128 * t
                    lhs = sel_s[:, m0 : m0 + pt]

                    of = outfp.tile([128, SECW], F32, tag="of")
                    for q, qw in pq:
                        psf = psp.tile([128, PSW], F32, tag="ps")
                        for k in range(0, qw, CW):
                            kw = min(CW, qw - k)
                            nc.tensor.matmul(
                                psf[:pt, k : k + kw],
                                lhs,
                                phi_s[:, q + k : q + k + kw],
                                start=True,
                                stop=False,
                            )
                            nc.tensor.matmul(
                                psf[:pt, k : k + kw],
                                lhs,
                                plo_s[:, q + k : q + k + kw],
                                start=False,
                                stop=True,
                            )
                        nc.vector.tensor_tensor(
                            of[:pt, q : q + qw],
                            psf[:pt, :qw],
                            isrep_s[:pt, q : q + qw],
                            OP.mult,
                        )
                    nc.sync.dma_start(out_f_d[m0 : m0 + pt, sec], of[:pt, :w])

                    ot = outtp.tile([128, SECW], F32, tag="ot")
                    for q, qw in pq:
                        pso = psp.tile([128, PSW], F32, tag="ps")
                        for k in range(0, qw, CW):
                            kw = min(CW, qw - k)
                            nc.tensor.matmul(
                                pso[:pt, k : k + kw],
                                lhs,
                                oh_s[:, q + k : q + k + kw],
                                start=True,
                                stop=True,
                            )
                        nc.vector.tensor_tensor(
                            ot[:pt, q : q + qw],
                            pso[:pt, :qw],
                            irep_s[:pt, q : q + qw],
                            OP.mult,
                        )
                    nc.scalar.dma_start(out_t_d[m0 : m0 + pt, sec], ot[:pt, :w])

            # software-pipelined emission: P0, P1, M0, P2, M1, P3, M2, M3
            states = [prologue(0)]
            for si in range(1, len(SECS)):
                states.append(prologue(si))
                main(states[si - 1])
            main(states[-1])

            # pooling epilogue: ref does (sum/64) / (cnt/64 + 1e-10)
            nc.vector.tensor_scalar(
                smacc, smacc, 1.0 / 64.0, 1e-10, OP.mult, OP.add
            )
            nc.vector.reciprocal(smacc, smacc)
            nc.vector.scalar_tensor_tensor(
                pooled_s, svacc, 1.0 / 64.0, smacc, OP.mult, OP.mult
            )
            nc.sync.dma_start(pooled_d, pooled_s)

    nc.finalize()
    return nc


_CACHE: dict = {}


def _get_program():
    if "nc" not in _CACHE:
        _CACHE["nc"] = build_program()
    return _CACHE["nc"]


def _make_in_maps(image_features, depth_logits, depth_maps, depth_target_bin):
    img = np.ascontiguousarray(np.asarray(image_features, np.float32)).reshape(
        B, C, HW
    )
    logits = np.ascontiguousarray(np.asarray(depth_logits, np.float32)).reshape(
        B, DP1, HW
    )
    binf16 = (
        np.asarray(depth_target_bin)
        .astype(np.float32)
        .reshape(B, 1, HW)
        .astype(ml_dtypes.bfloat16)
    )
    dmaps = np.ascontiguousarray(np.asarray(depth_maps, np.float32)).reshape(
        94, 8, 1248
    )

    onescolf = np.ones((DP1, 1), np.float32)
    negk = -np.arange(DP1, dtype=np.float32).reshape(DP1, 1)

    in_maps = []
    for core in range(NCORES):
        b, dc = divmod(core, 4)
        d0 = ND * dc
        # selection matrix: column m of d-tile t selects depth row d0+4t+m//32
        sel = np.zeros((DP1, ND * C), np.float32)
        for t in range(NT):
            pt = 128 if t < NT - 1 else 64
            for m in range(pt):
                k = d0 + 4 * t + m // 32
                sel[k, 128 * t + m] = 1.0
        in_maps.append(
            {
                "img": img[b],
                "logits": logits[b],
                "binf16": binf16[b],
                "negk": negk,
                "sel": sel.astype(ml_dtypes.bfloat16),
                "onescolf": onescolf,
                "dmaps": dmaps,
            }
        )
    return in_maps


def kernel(
    image_features,
    depth_logits,
    depth_maps,
    depth_target_bin,
    _trace=False,
    _tmpdir=None,
):
    nc = _get_program()
    in_maps = _make_in_maps(
        image_features, depth_logits, depth_maps, depth_target_bin
    )
    res = run_bass_kernel_spmd(
        nc,
        in_maps,
        core_ids=list(range(NCORES)),
        trace=_trace,
        tmpdir=_tmpdir,
    )
    _CACHE["last_results"] = res

    frustum = np.empty((B, C, D, H, W), np.float32)
    frustum_t = np.empty((B, C, D, H, W), np.float32)
    for core in range(NCORES):
        b, dc = divmod(core, 4)
        r = res.results[core]
        f = r["out_f"].reshape(ND, C, H, W).transpose(1, 0, 2, 3)
        ft = r["out_t"].reshape(ND, C, H, W).transpose(1, 0, 2, 3)
        frustum[b, :, ND * dc : ND * (dc + 1)] = f
        frustum_t[b, :, ND * dc : ND * (dc + 1)] = ft
    pooled = res.results[0]["pooled"].reshape(B, H, W).copy()
    return frustum, frustum_t, pooled


# revision 22
# speedup vs baseline: 1.0888x; 1.0888x over previous
"""Trainium2 Bass kernel for DepthFFN (histogram_binning).

Computes, for inputs
  image_features  (2, 32, 47, 156)  f32
  depth_logits    (2, 121, 47, 156) f32
  depth_maps      (2, 376, 1248)    f32
  depth_target_bin(2, 47, 156)      i32
the reference outputs
  frustum_features        (2, 32, 120, 47, 156) = softmax(logits)[:, :120] x image_features
  frustum_features_target (2, 32, 120, 47, 156) = onehot(bin)[:, :120]    x image_features
  pooled_depth            (2, 47, 156)          = sparse 8x8 avg pool of depth_maps

Sharding: 8 cores = (batch b in {0,1}) x (depth chunk dc in {0..3}, 30 bins
each). Each core writes its [30, 32, 7332] d-major slice of both frustum
tensors, and redundantly computes the (tiny) pooled output; the host takes
core 0's copy.

Structure: the hw = 7332 free axis is cut into 4 pipeline sections
(3 x 2048 + 1188); every step is column-local so sections overlap through
the Tile scheduler. All matmuls are bf16 (full PE rate):
  - exp = Exp(logits) is split into bf16 hi + lo parts (hi+lo carries
    ~17 mantissa bits; everything downstream accumulates in fp32 PSUM);
  - softmax sum = ones^T (exp_hi + exp_lo) (PE); exact reciprocal via a
    DRAM-bounce reshape onto ~64 partitions (DVE); the 1/sum row is folded
    into a scaled image (img * 1/sum broadcast over 32 partitions by a
    replicating DMA), never into the probs;
  - one-hot(bin) [121, w] built once per section by two ACT passes
    (Relu(1 - Abs(bin_row - k_partition))), exact 0/1 in bf16;
  - per d-tile (4 depth bins x 32 channels on 128 partitions): one bf16
    selection matmul replicates exp hi+lo rows (features) and one-hot rows
    (target) into PSUM; frustum tile = psum * img_scaled_rep (DVE);
    target tile = psum * img_rep, split between DVE (PSUM direct) and
    ACT-copy + GpSimd (SBUF) to balance engine load;
  - output stores alternate between the two HWDGE queues (Sync and Scalar
    sequencers) to halve per-queue DMA dispatch occupancy.
"""

import numpy as np
import ml_dtypes

import concourse.bacc as bacc
import concourse.bass as bass
import concourse.mybir as mybir
import concourse.tile as tile
from concourse.bass_utils import run_bass_kernel_spmd

F32 = mybir.dt.float32
BF16 = mybir.dt.bfloat16
AF = mybir.ActivationFunctionType
OP = mybir.AluOpType
AX = mybir.AxisListType

B, C, D, DP1 = 2, 32, 120, 121
H, W = 47, 156
HW = H * W  # 7332
ND = 30  # depth bins per core
NCORES = 8
CW = 512  # matmul moving-operand chunk (one fp32 PSUM bank)
PSW = 512  # PSUM tile width (1 bank -> 8 pipeline slots)
SECW = 2048  # pipeline section width
SECS = [(s, min(SECW, HW - s)) for s in range(0, HW, SECW)]
# per-section reshape factors for the reciprocal bounce (rows x cols = w)
RECIP_SHAPE = {2048: (64, 32), 1188: (44, 27)}
NT = 8  # d-tiles per core: 7 x (4 bins x 32 ch) + 1 x (2 bins x 32 ch)
PQ = 312  # pooling quarter width (in depth_maps columns)
# target-tile route: per (t, section), True -> GpSimd path, False -> DVE path.
# ~3/4 on GpSimd frees the DVE for the frustum stream.
GP_ROUTE = [t % 2 == 0 for t in range(NT)]


def build_program():
    nc = bacc.Bacc(
        "TRN2",
        target_bir_lowering=False,
        debug=False,
        num_devices=NCORES,
    )

    img_d = nc.dram_tensor("img", [C, HW], F32, kind="ExternalInput").ap()
    logits_d = nc.dram_tensor("logits", [DP1, HW], F32, kind="ExternalInput").ap()
    binf16_d = nc.dram_tensor("binf16", [1, HW], BF16, kind="ExternalInput").ap()
    negk_d = nc.dram_tensor("negk", [DP1, 1], F32, kind="ExternalInput").ap()
    sel_d = nc.dram_tensor("sel", [DP1, ND * C], BF16, kind="ExternalInput").ap()
    onescolf_d = nc.dram_tensor("onescolf", [DP1, 1], F32, kind="ExternalInput").ap()
    dmaps_d = nc.dram_tensor("dmaps", [94, 8, 1248], F32, kind="ExternalInput").ap()

    out_f_d = nc.dram_tensor("out_f", [ND * C, HW], F32, kind="ExternalOutput").ap()
    out_t_d = nc.dram_tensor("out_t", [ND * C, HW], F32, kind="ExternalOutput").ap()
    pooled_d = nc.dram_tensor("pooled", [94, 156], F32, kind="ExternalOutput").ap()

    with tile.TileContext(nc) as tc:
        with (
            tc.tile_pool(name="const", bufs=1) as constp,
            tc.tile_pool(name="sec", bufs=2) as secp,
            tc.tile_pool(name="outf", bufs=4) as outfp,
            tc.tile_pool(name="outt", bufs=4) as outtp,
            tc.tile_pool(name="psum", bufs=8, space="PSUM") as psp,
            tc.tile_pool(name="dram", bufs=1, space="DRAM") as dramp,
            tc.tile_pool(name="poolx", bufs=1) as poolxp,
        ):
            # ---- constants ----
            sel_s = constp.tile([DP1, ND * C], BF16, tag="sel")
            nc.sync.dma_start(sel_s, sel_d)
            onescolf_s = constp.tile([DP1, 1], F32, tag="onescolf")
            nc.sync.dma_start(onescolf_s, onescolf_d)
            negk_s = constp.tile([DP1, 1], F32, tag="negk")
            nc.sync.dma_start(negk_s, negk_d)

            sums_b = dramp.tile([1, HW], F32, tag="sums")
            inv_b = dramp.tile([1, HW], F32, tag="inv")

            pooled_s = poolxp.tile([94, 156], F32, tag="pooled")
            svacc = poolxp.tile([94, 156], F32, tag="svacc")
            smacc = poolxp.tile([94, 156], F32, tag="smacc")

            def prologue(si):
                c0, w = SECS[si]
                sec = slice(c0, c0 + w)
                pq = [(q, min(PSW, w - q)) for q in range(0, w, PSW)]

                # ---- prologue: exp, hi/lo split, softmax sum, reciprocal ----
                exp_s = secp.tile([DP1, SECW], F32, tag="exp")
                nc.sync.dma_start(exp_s[:, :w], logits_d[:, sec])
                nc.scalar.activation(exp_s[:, :w], exp_s[:, :w], AF.Exp)
                phi_s = secp.tile([DP1, SECW], BF16, tag="phi")
                nc.vector.tensor_copy(phi_s[:, :w], exp_s[:, :w])
                plo_s = secp.tile([DP1, SECW], BF16, tag="plo")
                nc.vector.tensor_tensor(
                    plo_s[:, :w], exp_s[:, :w], phi_s[:, :w], OP.subtract
                )

                sum_s = secp.tile([1, SECW], F32, tag="sumsec", bufs=1)
                for q, qw in pq:
                    ps = psp.tile([128, PSW], F32, tag="ps")
                    for k in range(0, qw, CW):
                        kw = min(CW, qw - k)
                        nc.tensor.matmul(
                            ps[:1, k : k + kw],
                            onescolf_s,
                            exp_s[:, q + k : q + k + kw],
                            start=True,
                            stop=True,
                        )
                    nc.scalar.copy(sum_s[:, q : q + qw], ps[:1, :qw])
                nc.sync.dma_start(sums_b[:, sec], sum_s[:, :w])

                rr, rc = RECIP_SHAPE[w]
                r_s = secp.tile([rr, rc], F32, tag="rsec")
                nc.sync.dma_start(
                    r_s, sums_b[:, sec].rearrange("o (p q) -> (o p) q", q=rc)
                )
                nc.vector.reciprocal(r_s, r_s)
                nc.sync.dma_start(
                    inv_b[:, sec].rearrange("o (p q) -> (o p) q", q=rc), r_s
                )

                # ---- image: raw + softmax-scaled, each replicated 4x,
                # built directly inside the replicated tiles ----
                irep_s = secp.tile([128, SECW], F32, tag="irep")
                isrep_s = secp.tile([128, SECW], F32, tag="isrep")
                nc.sync.dma_start(irep_s[:C, :w], img_d[:, sec])
                nc.sync.dma_start(
                    isrep_s[:C, :w],
                    bass.AP(inv_b.tensor, inv_b.offset + c0, [[0, C], [1, w]]),
                )
                nc.vector.tensor_tensor(
                    isrep_s[:C, :w], isrep_s[:C, :w], irep_s[:C, :w], OP.mult
                )
                for k in range(1, 4):
                    nc.sync.dma_start(
                        irep_s[32 * k : 32 * (k + 1), :w], irep_s[:C, :w]
                    )
                    nc.sync.dma_start(
                        isrep_s[32 * k : 32 * (k + 1), :w], isrep_s[:C, :w]
                    )

                # ---- one-hot(bin) [121, w], exact 0/1 in bf16 ----
                brep_s = secp.tile([DP1, SECW], BF16, tag="brep", bufs=1)
                nc.scalar.dma_start(
                    brep_s[:, :w],
                    bass.AP(
                        binf16_d.tensor, binf16_d.offset + c0, [[0, DP1], [1, w]]
                    ),
                )
                oh_s = secp.tile([DP1, SECW], BF16, tag="oh")
                # |bin - k|, then relu(1 - |bin - k|): 1 iff bin == k
                nc.scalar.activation(
                    oh_s[:, :w], brep_s[:, :w], AF.Abs, bias=negk_s, scale=1.0
                )
                nc.scalar.activation(
                    oh_s[:, :w], oh_s[:, :w], AF.Relu, bias=1.0, scale=-1.0
                )
                # ---- pooling quarter: rows [2*si, 2*si+2) of each 8x8
                # block (contiguous 2x1248 floats per partition), accumulated
                # across sections ----
                xp = poolxp.tile([94, 2, 1248], F32, tag="xp", bufs=1)
                nc.sync.dma_start(xp, dmaps_d[:, 2 * si : 2 * si + 2, :])
                mkp = poolxp.tile([94, 2, 1248], F32, tag="mkp", bufs=1)
                # depth values are >= 0, so Sign(x) == (x != 0)
                nc.scalar.activation(mkp, xp, AF.Sign)
                svq = svacc if si == 0 else poolxp.tile(
                    [94, 156], F32, tag="svq", bufs=2
                )
                smq = smacc if si == 0 else poolxp.tile(
                    [94, 156], F32, tag="smq", bufs=2
                )
                nc.vector.tensor_reduce(
                    svq,
                    xp.rearrange("p r (j q) -> p j r q", q=8),
                    axis=AX.XY,
                    op=OP.add,
                )
                nc.vector.tensor_reduce(
                    smq,
                    mkp.rearrange("p r (j q) -> p j r q", q=8),
                    axis=AX.XY,
                    op=OP.add,
                )
                if si > 0:
                    nc.vector.tensor_add(svacc, svacc, svq)
                    nc.vector.tensor_add(smacc, smacc, smq)

                return dict(
                    c0=c0, w=w, sec=sec, pq=pq, phi=phi_s, plo=plo_s,
                    irep=irep_s, isrep=isrep_s, oh=oh_s
                )

            def main(st):
                sec, pq, w = st["sec"], st["pq"], st["w"]
                phi_s, plo_s = st["phi"], st["plo"]
                irep_s, isrep_s, oh_s = st["irep"], st["isrep"], st["oh"]
                for t in range(NT):
                    pt = 128 if t < NT - 1 else 64
                    m0 = 128 * t
                    lhs = sel_s[:, m0 : m0 + pt]

                    of = outfp.tile([128, SECW], F32, tag="of")
                    for q, qw in pq:
                        psf = psp.tile([128, PSW], F32, tag="ps")
                        for k in range(0, qw, CW):
                            kw = min(CW, qw - k)
                            nc.tensor.matmul(
                                psf[:pt, k : k + kw],
                                lhs,
                                phi_s[:, q + k : q + k + kw],
                                start=True,
                                stop=False,
                            )
                            nc.tensor.matmul(
                                psf[:pt, k : k + kw],
                                lhs,
                                plo_s[:, q + k : q + k + kw],
                                start=False,
                                stop=True,
                            )
                        nc.vector.tensor_tensor(
                            of[:pt, q : q + qw],
                            psf[:pt, :qw],
                            isrep_s[:pt, q : q + qw],
                            OP.mult,
                        )
                    nc.sync.dma_start(out_f_d[m0 : m0 + pt, sec], of[:pt, :w])

                    ot = outtp.tile([128, SECW], F32, tag="ot")
                    for q, qw in pq:
                        pso = psp.tile([128, PSW], F32, tag="ps")
                        for k in range(0, qw, CW):
                            kw = min(CW, qw - k)
                            nc.tensor.matmul(
                                pso[:pt, k : k + kw],
                                lhs,
                                oh_s[:, q + k : q + k + kw],
                                start=True,
                                stop=True,
                            )
                        nc.vector.tensor_tensor(
                            ot[:pt, q : q + qw],
                            pso[:pt, :qw],
                            irep_s[:pt, q : q + qw],
                            OP.mult,
                        )
                    nc.scalar.dma_start(out_t_d[m0 : m0 + pt, sec], ot[:pt, :w])

            # software-pipelined emission: P0, P1, M0, P2, M1, P3, M2, M3
            states = [prologue(0)]
            for si in range(1, len(SECS)):
                states.append(prologue(si))
                main(states[si - 1])
            main(states[-1])

            # pooling epilogue: ref does (sum/64) / (cnt/64 + 1e-10)
            nc.vector.tensor_scalar(
                smacc, smacc, 1.0 / 64.0, 1e-10, OP.mult, OP.add
            )
            nc.vector.reciprocal(smacc, smacc)
            nc.vector.scalar_tensor_tensor(
                pooled_s, svacc, 1.0 / 64.0, smacc, OP.mult, OP.mult
            )
            nc.sync.dma_start(pooled_d, pooled_s)

    nc.finalize()
    return nc


_CACHE: dict = {}


def _get_program():
    if "nc" not in _CACHE:
        _CACHE["nc"] = build_program()
    return _CACHE["nc"]


def _make_in_maps(image_features, depth_logits, depth_maps, depth_target_bin):
    img = np.ascontiguousarray(np.asarray(image_features, np.float32)).reshape(
        B, C, HW
    )
    logits = np.ascontiguousarray(np.asarray(depth_logits, np.float32)).reshape(
        B, DP1, HW
    )
    binf16 = (
        np.asarray(depth_target_bin)
        .astype(np.float32)
        .reshape(B, 1, HW)
        .astype(ml_dtypes.bfloat16)
    )
    dmaps = np.ascontiguousarray(np.asarray(depth_maps, np.float32)).reshape(
        94, 8, 1248
    )

    onescolf = np.ones((DP1, 1), np.float32)
    negk = -np.arange(DP1, dtype=np.float32).reshape(DP1, 1)

    in_maps = []
    for core in range(NCORES):
        b, dc = divmod(core, 4)
        d0 = ND * dc
        # selection matrix: column m of d-tile t selects depth row d0+4t+m//32
        sel = np.zeros((DP1, ND * C), np.float32)
        for t in range(NT):
            pt = 128 if t < NT - 1 else 64
            for m in range(pt):
                k = d0 + 4 * t + m // 32
                sel[k, 128 * t + m] = 1.0
        in_maps.append(
            {
                "img": img[b],
                "logits": logits[b],
                "binf16": binf16[b],
                "negk": negk,
                "sel": sel.astype(ml_dtypes.bfloat16),
                "onescolf": onescolf,
                "dmaps": dmaps,
            }
        )
    return in_maps


def kernel(
    image_features,
    depth_logits,
    depth_maps,
    depth_target_bin,
    _trace=False,
    _tmpdir=None,
):
    nc = _get_program()
    in_maps = _make_in_maps(
        image_features, depth_logits, depth_maps, depth_target_bin
    )
    res = run_bass_kernel_spmd(
        nc,
        in_maps,
        core_ids=list(range(NCORES)),
        trace=_trace,
        tmpdir=_tmpdir,
    )
    _CACHE["last_results"] = res

    frustum = np.empty((B, C, D, H, W), np.float32)
    frustum_t = np.empty((B, C, D, H, W), np.float32)
    for core in range(NCORES):
        b, dc = divmod(core, 4)
        r = res.results[core]
        f = r["out_f"].reshape(ND, C, H, W).transpose(1, 0, 2, 3)
        ft = r["out_t"].reshape(ND, C, H, W).transpose(1, 0, 2, 3)
        frustum[b, :, ND * dc : ND * (dc + 1)] = f
        frustum_t[b, :, ND * dc : ND * (dc + 1)] = ft
    pooled = res.results[0]["pooled"].reshape(B, H, W).copy()
    return frustum, frustum_t, pooled
